# revision 22
# baseline (speedup 1.0000x reference)
"""Trainium2 Bass kernel for nn_BaseAttention_13795434955497.

The reference module is a "linear attention" whose einsum reductions are all
over the head-depth axis only (bhld->bhl), so every token is independent:

    q   = elu(query @ Wq) + 1            [B,H,L,D]
    k   = elu(key   @ Wk) + 1
    v   = value @ Wv
    ks  = sum_d k                        [B,H,L]
    wv  = sum_d k*v                      [B,H,L]
    ctx = q*wv / (q*ks + 1e-6)           [B,H,L,D]
    out = LN(query + ctx @ Wo)

With q > 0 and ks ~ 40..110 the epsilon is ~1e-5 relative, so
ctx[., h, d] == (wv/ks)[., h] independent of q and d: the q projection is
never needed and ctx @ Wo == r @ Wo_red with Wo_red[h,:] = sum_d Wo[64h+d,:].

Token-parallel over B*L = 16384 tokens across 8 NeuronCores, no collectives.
Host-side sharding also pre-packs the weights (Wk/Wv cast to bf16, Wo reduced
to the rank-16 Wo_red) so each core reads 4 MiB of weights instead of 12.

Per-core dataflow, fully software-pipelined at 128-token subtile granularity
so the PE never idles (HAM throttle stays warm) and no serialized staging
phase exists:

  gpsimd/SWDGE : cast-load xk/xv fp32->bf16 HBM->SBUF, 512-token blocks
  PE           : 128x128 transposes of k/v subtiles (token-major -> d-major),
                 k/v projections (bf16, fp32 accum), rank-16 attn matmul
  ACT          : exp(k), k+1, PSUM->SBUF copies, Square+accum for LN,
                 rstd = exp(-0.5*ln(var+eps))  [single table set: ln+exp],
                 final (x-mean)*rstd via Identity(scale,bias) APs
  DVE          : elu combine (bf16 2x), k*v, per-head reduces, reciprocal,
                 residual add + mean accum, small LN chain
  sync/HWDGE   : weight loads, xq loads, output stores

PSUM budget (8 banks): 2x proj [128,1024]f32 (4) + transpose staging
[128,8,128]bf16 (1) + attn [128,1024]f32 (2) + rT [16,128]bf16 (1).
"""

import numpy as np
from contextlib import ExitStack

import concourse.bass as bass
import concourse.tile as tile
from concourse import bacc, mybir
from concourse.bass_utils import run_bass_kernel_spmd
from concourse.masks import make_identity

F32 = mybir.dt.float32
BF16 = mybir.dt.bfloat16
AF = mybir.ActivationFunctionType
OP = mybir.AluOpType
AX = mybir.AxisListType

N_CORES = 8
B, L, DM, H = 4, 4096, 1024, 16
D = DM // H                      # 64
NTOK = B * L                     # 16384
TOK = NTOK // N_CORES            # 2048 tokens per core
NCH = DM // 128                  # 8 contraction chunks
NSUB = TOK // 128                # 16 token subtiles per core
SUB_BLK = 4                      # subtiles per DMA block (512 tokens)
NBLK = NSUB // SUB_BLK
EPS_LN = 1e-3
RSQRT_MAGIC = 0x5F3759DF
I32 = mybir.dt.int32

# fp8e4m3 projections with DoubleRow (2 contraction rows per PE cell,
# ~1.7x over bf16).  Weights are pre-scaled by W_SCALE on the host so they
# sit in e4m3's normal range; the activation reads descale for free via the
# ACT affine (scale=1/W_SCALE).
USE_FP8 = False
FP8 = mybir.dt.float8e4
W_SCALE = 32.0


def _build_core_program():
    nc = bacc.Bacc(
        "TRN2",
        target_bir_lowering=False,
        debug=False,
        enable_asserts=False,
        num_devices=N_CORES,
    )
    WDT = FP8 if USE_FP8 else BF16
    xq = nc.dram_tensor("xq", [TOK, DM], F32, kind="ExternalInput").ap()
    xk = nc.dram_tensor("xk", [TOK, DM], F32, kind="ExternalInput").ap()
    xv = nc.dram_tensor("xv", [TOK, DM], F32, kind="ExternalInput").ap()
    wk = nc.dram_tensor("wk", [DM, DM], WDT, kind="ExternalInput").ap()
    wv = nc.dram_tensor("wv", [DM, DM], WDT, kind="ExternalInput").ap()
    wored = nc.dram_tensor("wored", [H, DM], BF16, kind="ExternalInput").ap()
    out = nc.dram_tensor("out", [TOK, DM], F32, kind="ExternalOutput").ap()

    with tile.TileContext(nc) as tc:
        with ExitStack() as ctx:
            _emit(ctx, tc, xq, xk, xv, wk, wv, wored, out)

    nc.compile()
    return nc


def _emit(ctx, tc, xq, xk, xv, wk, wv, wored, out):
    nc = tc.nc

    const = ctx.enter_context(tc.tile_pool(name="const", bufs=1))
    wpool = ctx.enter_context(tc.tile_pool(name="w", bufs=1))
    xblk = ctx.enter_context(tc.tile_pool(name="xblk", bufs=3))
    xtp = ctx.enter_context(tc.tile_pool(name="xt", bufs=2))
    qp = ctx.enter_context(tc.tile_pool(name="q32", bufs=3))
    tmpb = ctx.enter_context(tc.tile_pool(name="tmpb", bufs=2))
    small = ctx.enter_context(tc.tile_pool(name="small", bufs=4))
    xresp = ctx.enter_context(tc.tile_pool(name="xres", bufs=2))
    outp = ctx.enter_context(tc.tile_pool(name="outp", bufs=3))
    ps_proj = ctx.enter_context(tc.tile_pool(name="ps_proj", bufs=2, space="PSUM"))
    ps_t = ctx.enter_context(tc.tile_pool(name="ps_t", bufs=1, space="PSUM"))
    ps_attn = ctx.enter_context(tc.tile_pool(name="ps_attn", bufs=1, space="PSUM"))
    ps_rt = ctx.enter_context(tc.tile_pool(name="ps_rt", bufs=1, space="PSUM"))

    ident = const.tile([128, 128], BF16)
    make_identity(nc, ident)

    # Constants for activation bias APs and the Newton iteration.
    cvals = [0.0, 1.0, 1.5]
    ctile = const.tile([128, len(cvals)], F32)
    for i, v in enumerate(cvals):
        nc.vector.memset(ctile[:, i : i + 1], v)
        nc.const_aps.aps[(F32, v)] = ctile[:, i : i + 1]
    c_1p5 = ctile[:, 2:3]

    xsrc = {"k": xk, "v": xv}
    state = {}

    def s_load(m, name):
        # Per-subtile SWDGE cast-load: small units keep the HBM draw smooth
        # and let the startup-critical transfers finish first (the SDMA
        # engines round-robin between rings at packet granularity, so one
        # huge early transfer starves the critical weight load).
        t = xblk.tile([128, DM], BF16, tag=f"x{name}s")
        nc.gpsimd.dma_start(out=t, in_=xsrc[name][m * 128 : (m + 1) * 128, :])
        state[(name, "tok", m)] = t

    def s_transpose(m, name):
        # 8 PE transposes of one subtile into one PSUM bank, one copy out.
        src = state.pop((name, "tok", m))
        pst = ps_t.tile([128, NCH, 128], BF16, tag="pst")
        for c in range(NCH):
            nc.tensor.transpose(
                pst[:, c, :], src[:, c * 128 : (c + 1) * 128], ident
            )
        xT = xtp.tile([128, NCH, 128], FP8 if USE_FP8 else BF16, tag=f"{name}T")
        if name == "k":
            nc.scalar.copy(xT, pst)
        else:
            nc.vector.tensor_scalar(
                out=xT, in0=pst, scalar1=0.0, scalar2=None, op0=OP.add
            )
        state[(name, "xT", m)] = xT

    def s_proj(m, name):
        xT = state.pop((name, "xT", m))
        w_sb = wk_sb if name == "k" else wv_sb
        p = ps_proj.tile([128, DM], F32, tag="proj")
        if USE_FP8:
            # DoubleRow: both operands supply chunk-PAIRS [128, 2, n].
            for c2 in range(NCH // 2):
                for h in range(2):
                    nc.tensor.matmul(
                        p[:, h * 512 : (h + 1) * 512],
                        lhsT=xT[:, 2 * c2 : 2 * c2 + 2, :],
                        rhs=w_sb[:, 2 * c2 : 2 * c2 + 2, h * 512 : (h + 1) * 512],
                        start=(c2 == 0),
                        stop=(c2 == NCH // 2 - 1),
                        perf_mode=mybir.MatmulPerfMode.DoubleRow,
                    )
        else:
            for c in range(NCH):
                for h in range(2):
                    nc.tensor.matmul(
                        p[:, h * 512 : (h + 1) * 512],
                        lhsT=xT[:, c, :],
                        rhs=w_sb[:, c, h * 512 : (h + 1) * 512],
                        start=(c == 0),
                        stop=(c == NCH - 1),
                    )
        state[(name, "ps", m)] = p

    def s_eluk(m):
        psk = state.pop(("k", "ps", m))
        psv = state.pop(("v", "ps", m))
        # elu(k)+1 == max(min(exp(k),1), k+1); bf16 intermediates for DVE 2x.
        # The 1/W_SCALE descale of the fp8 weight pre-scaling rides the free
        # ACT affine on every PSUM read.
        ds = 1.0 / W_SCALE if USE_FP8 else 1.0
        ek = tmpb.tile([128, DM], BF16, tag="ek")
        nc.scalar.activation(ek, psk, AF.Exp, scale=ds)
        k1 = tmpb.tile([128, DM], BF16, tag="k1")
        nc.scalar.activation(k1, psk, AF.Identity, bias=1.0, scale=ds)
        vb = tmpb.tile([128, DM], BF16, tag="vb")
        nc.scalar.mul(vb, psv, ds)
        kf = tmpb.tile([128, DM], BF16, tag="kf")
        nc.vector.scalar_tensor_tensor(
            out=kf, in0=ek, scalar=1.0, in1=k1, op0=OP.min, op1=OP.max
        )
        # k*v and the per-head reductions run as bf16 tree-adds on the
        # (otherwise idle) gpsimd engine; the DVE TensorReduce has no 2x
        # uop so it costs 1.2us per call there.  Only the final 8->1
        # fp32-accumulating step stays on the DVE.
        kv = tmpb.tile([128, DM], BF16, tag="kv")
        nc.gpsimd.tensor_tensor(kv, kf, vb, op=OP.mult)

        def tree(src, tag):
            t1 = tmpb.tile([128, H, D // 2], BF16, tag=f"{tag}1")
            s3 = src.rearrange("p (h d) -> p h d", h=H)
            nc.gpsimd.tensor_tensor(
                t1, s3[:, :, 0 : D // 2], s3[:, :, D // 2 : D], op=OP.add
            )
            t2 = tmpb.tile([128, H, D // 4], BF16, tag=f"{tag}2")
            nc.gpsimd.tensor_tensor(
                t2, t1[:, :, 0 : D // 4], t1[:, :, D // 4 : D // 2], op=OP.add
            )
            t3 = tmpb.tile([128, H, D // 8], BF16, tag=f"{tag}3")
            nc.gpsimd.tensor_tensor(
                t3, t2[:, :, 0 : D // 8], t2[:, :, D // 8 : D // 4], op=OP.add
            )
            out = small.tile([128, H], F32, tag=f"{tag}s")
            nc.vector.reduce_sum(out, t3, axis=AX.X)
            return out

        ks = tree(kf, "ks")
        wvs = tree(kv, "wv")
        rk = small.tile([128, H], F32, tag="rk")
        nc.vector.reciprocal(rk, ks)
        r = small.tile([128, H], BF16, tag="r")
        nc.vector.tensor_mul(r, wvs, rk)
        state[("r", m)] = r

    def s_rT(m):
        r = state.pop(("r", m))
        rT_ps = ps_rt.tile([16, 128], BF16, tag="rt")
        nc.tensor.transpose(rT_ps, r, ident)
        rT = small.tile([16, 128], BF16, tag="rT")
        nc.scalar.copy(rT, rT_ps)
        state[("rT", m)] = rT

    def s_attn(m):
        rT = state.pop(("rT", m))
        ap_ps = ps_attn.tile([128, DM], F32, tag="attn")
        for h in range(2):
            nc.tensor.matmul(
                ap_ps[:, h * 512 : (h + 1) * 512],
                lhsT=rT,
                rhs=wo_sb[:, h * 512 : (h + 1) * 512],
                start=True,
                stop=True,
            )
        state[("attn", m)] = ap_ps

    def s_qload(m):
        q32 = qp.tile([128, DM], F32, tag="q32")
        nc.sync.dma_start(out=q32, in_=xq[m * 128 : (m + 1) * 128, :])
        state[("q32", m)] = q32

    def s_ln(m):
        ap_ps = state.pop(("attn", m))
        q32 = state.pop(("q32", m))
        # Residual add; row-sum (-> mean) rides along via accum_out.
        xres = xresp.tile([128, DM], F32, tag="xres")
        sx = small.tile([128, 2], F32, tag="sx")
        nc.vector.scalar_tensor_tensor(
            out=xres,
            in0=ap_ps,
            scalar=0.0,
            in1=q32,
            op0=OP.add,
            op1=OP.add,
            accum_out=sx[:, 0:1],
        )
        xsq = tmpb.tile([128, DM], BF16, tag="xsq")
        nc.scalar.activation(xsq, xres, AF.Square, accum_out=sx[:, 1:2])
        mv = small.tile([128, 2], F32, tag="mv")
        nc.vector.tensor_scalar(
            out=mv, in0=sx, scalar1=1.0 / DM, scalar2=None, op0=OP.mult
        )
        # rstd = rsqrt(var + eps): bit-trick seed + 2 Newton steps, DVE only
        # (the Sqrt/Ln ACT tables live in different table sets than Exp, and
        # a table-set switch costs ~2.7us -- never load anything but Exp).
        nwt = small.tile([128, 10], F32, tag="nwt")
        ve = nwt[:, 0:1]
        nc.vector.tensor_scalar(
            out=ve, in0=mv[:, 1:2], scalar1=EPS_LN, scalar2=None, op0=OP.add
        )
        mneg = nwt[:, 1:2]
        nc.vector.tensor_scalar(
            out=mneg, in0=mv[:, 0:1], scalar1=-1.0, scalar2=None, op0=OP.mult
        )
        v1 = nwt[:, 2:3]
        nc.vector.scalar_tensor_tensor(
            out=v1, in0=mneg, scalar=mv[:, 0:1], op0=OP.mult, in1=ve, op1=OP.add
        )
        hx = nwt[:, 3:4]
        nc.vector.tensor_scalar(
            out=hx, in0=v1, scalar1=0.5, scalar2=None, op0=OP.mult
        )
        sshift = nwt[:, 4:5].bitcast(I32)
        nc.vector.tensor_scalar(
            out=sshift,
            in0=v1.bitcast(I32),
            scalar1=1,
            scalar2=None,
            op0=OP.arith_shift_right,
        )
        # magic - s == (s ^ 0xffffffff) + (magic + 1)  (int32 wraparound)
        nc.vector.tensor_scalar(
            out=sshift, in0=sshift, scalar1=-1, scalar2=None, op0=OP.bitwise_xor
        )
        y = nwt[:, 5:6]
        nc.vector.tensor_scalar(
            out=y.bitcast(I32),
            in0=sshift,
            scalar1=RSQRT_MAGIC + 1,
            scalar2=None,
            op0=OP.add,
        )
        for it in range(2):
            yy = nwt[:, 6:7]
            nc.vector.tensor_mul(yy, y, y)
            t = nwt[:, 7:8]
            # t = yy*hx - 1.5 ; z = y*t = -Newton(y); two steps restore sign
            nc.vector.scalar_tensor_tensor(
                out=t, in0=yy, scalar=hx, in1=c_1p5, op0=OP.mult, op1=OP.subtract
            )
            z = nwt[:, 8 + it : 9 + it]
            nc.vector.tensor_mul(z, y, t)
            y = z
        rstd = y
        nb = nwt[:, 7:8]
        nc.vector.tensor_scalar(
            out=nb, in0=mv[:, 0:1], scalar1=-1.0, scalar2=rstd, op0=OP.mult, op1=OP.mult
        )
        o = outp.tile([128, DM], F32, tag="o")
        nc.scalar.activation(o, xres, AF.Identity, bias=nb, scale=rstd)
        nc.sync.dma_start(out=out[m * 128 : (m + 1) * 128, :], in_=o)

    # Prime with the startup-critical transfers first: subtile 0's k/v casts
    # and the weight loads, split across the three DMA-issuing rings
    # (gpsimd SWDGE / sync HWDGE / scalar HWDGE) so they share HBM evenly.
    WDT = FP8 if USE_FP8 else BF16
    s_load(0, "k")
    wk_sb = wpool.tile([128, NCH, DM], WDT, tag="wk")
    nc.sync.dma_start(out=wk_sb, in_=wk.rearrange("(c p) j -> p c j", p=128))
    s_load(0, "v")
    wv_sb = wpool.tile([128, NCH, DM], WDT, tag="wv")
    nc.scalar.dma_start(out=wv_sb, in_=wv.rearrange("(c p) j -> p c j", p=128))
    wo_sb = wpool.tile([H, DM], BF16, tag="wo")
    nc.sync.dma_start(out=wo_sb, in_=wored)
    s_load(1, "k")
    s_load(1, "v")

    # Software pipeline.  PE queue order per tick m:
    #   Tk(m+1) Pk(m) rT(m-1) Tv(m+1) Pv(m) attn(m-1)
    # -- transposes for the next subtile are interleaved between this
    # subtile's projections so the shared PSUM staging bank alternates k/v
    # with the drain copies hidden under projection matmuls.
    for m in range(-1, NSUB + 1):
        if 0 <= m + 2 < NSUB:
            s_load(m + 2, "k")
            s_load(m + 2, "v")
        if 0 <= m + 1 < NSUB:
            s_transpose(m + 1, "k")
        if 0 <= m < NSUB:
            s_proj(m, "k")
            s_qload(m)
        if 0 <= m - 1 < NSUB:
            s_rT(m - 1)
        if 0 <= m + 1 < NSUB:
            s_transpose(m + 1, "v")
        if 0 <= m < NSUB:
            s_proj(m, "v")
        if 0 <= m - 1 < NSUB:
            s_attn(m - 1)
        if 0 <= m < NSUB:
            s_eluk(m)
        if 0 <= m - 1 < NSUB:
            s_ln(m - 1)


_NC_CACHE = None


def _get_program():
    global _NC_CACHE
    if _NC_CACHE is None:
        _NC_CACHE = _build_core_program()
    return _NC_CACHE


def _prep_weights(inputs):
    import ml_dtypes

    Wk = np.ascontiguousarray(np.asarray(inputs["Wk"], np.float32))
    Wv = np.ascontiguousarray(np.asarray(inputs["Wv"], np.float32))
    Wo = np.ascontiguousarray(np.asarray(inputs["Wo"], np.float32))
    if USE_FP8:
        wdt = ml_dtypes.float8_e4m3
        wk_c = (Wk * W_SCALE).astype(wdt)
        wv_c = (Wv * W_SCALE).astype(wdt)
    else:
        wk_c = Wk.astype(ml_dtypes.bfloat16)
        wv_c = Wv.astype(ml_dtypes.bfloat16)
    wored = Wo.reshape(H, D, DM).sum(axis=1).astype(ml_dtypes.bfloat16)
    return wk_c, wv_c, wored


def _make_in_maps(inputs):
    q = np.ascontiguousarray(np.asarray(inputs["query"], np.float32)).reshape(NTOK, DM)
    k = np.ascontiguousarray(np.asarray(inputs["key"], np.float32)).reshape(NTOK, DM)
    v = np.ascontiguousarray(np.asarray(inputs["value"], np.float32)).reshape(NTOK, DM)
    wk_c, wv_c, wored = _prep_weights(inputs)

    in_maps = []
    for i in range(N_CORES):
        sl = slice(i * TOK, (i + 1) * TOK)
        in_maps.append(
            {
                "xq": np.ascontiguousarray(q[sl]),
                "xk": np.ascontiguousarray(k[sl]),
                "xv": np.ascontiguousarray(v[sl]),
                "wk": wk_c,
                "wv": wv_c,
                "wored": wored,
            }
        )
    return in_maps


def kernel(**inputs) -> np.ndarray:
    nc = _get_program()
    in_maps = _make_in_maps(inputs)
    res = run_bass_kernel_spmd(nc, in_maps, core_ids=list(range(N_CORES)))
    full = np.concatenate([r["out"] for r in res.results], axis=0)
    return full.reshape(B, L, DM).astype(np.float32)


# revision 28
# speedup vs baseline: 1.3648x; 1.3648x over previous
"""Trainium2 Bass kernel for nn_BaseAttention_13795434955497.

The reference module is a "linear attention" whose einsum reductions are all
over the head-depth axis only (bhld->bhl), so every token is independent:

    q   = elu(query @ Wq) + 1            [B,H,L,D]
    k   = elu(key   @ Wk) + 1
    v   = value @ Wv
    ks  = sum_d k                        [B,H,L]
    wv  = sum_d k*v                      [B,H,L]
    ctx = q*wv / (q*ks + 1e-6)           [B,H,L,D]
    out = LN(query + ctx @ Wo)

With q > 0 and ks ~ 40..110 the epsilon is ~1e-5 relative, so
ctx[., h, d] == (wv/ks)[., h] independent of q and d: the q projection is
never needed and ctx @ Wo == r @ Wo_red with Wo_red[h,:] = sum_d Wo[64h+d,:].

Token-parallel over B*L = 16384 tokens across 8 NeuronCores, no collectives.
Host-side sharding also pre-packs the weights (Wk/Wv cast to bf16, Wo reduced
to the rank-16 Wo_red) so each core reads 4 MiB of weights instead of 12.

Per-core dataflow, fully software-pipelined at 128-token subtile granularity
so the PE never idles (HAM throttle stays warm) and no serialized staging
phase exists:

  gpsimd/SWDGE : cast-load xk/xv fp32->bf16 HBM->SBUF, 512-token blocks
  PE           : 128x128 transposes of k/v subtiles (token-major -> d-major),
                 k/v projections (bf16, fp32 accum), rank-16 attn matmul
  ACT          : exp(k), k+1, PSUM->SBUF copies, Square+accum for LN,
                 rstd = exp(-0.5*ln(var+eps))  [single table set: ln+exp],
                 final (x-mean)*rstd via Identity(scale,bias) APs
  DVE          : elu combine (bf16 2x), k*v, per-head reduces, reciprocal,
                 residual add + mean accum, small LN chain
  sync/HWDGE   : weight loads, xq loads, output stores

PSUM budget (8 banks): 2x proj [128,1024]f32 (4) + transpose staging
[128,8,128]bf16 (1) + attn [128,1024]f32 (2) + rT [16,128]bf16 (1).
"""

import numpy as np
from contextlib import ExitStack

import concourse.bass as bass
import concourse.tile as tile
from concourse import bacc, mybir
from concourse.bass_utils import run_bass_kernel_spmd
from concourse.masks import make_identity

F32 = mybir.dt.float32
BF16 = mybir.dt.bfloat16
AF = mybir.ActivationFunctionType
OP = mybir.AluOpType
AX = mybir.AxisListType

N_CORES = 8
B, L, DM, H = 4, 4096, 1024, 16
D = DM // H                      # 64
NTOK = B * L                     # 16384
TOK = NTOK // N_CORES            # 2048 tokens per core
NCH = DM // 128                  # 8 contraction chunks
NSUB = TOK // 128                # 16 token subtiles per core
SUB_BLK = 4                      # subtiles per DMA block (512 tokens)
NBLK = NSUB // SUB_BLK
EPS_LN = 1e-3
RSQRT_MAGIC = 0x5F3759DF
I32 = mybir.dt.int32

# fp8e4m3 projections with DoubleRow (2 contraction rows per PE cell,
# ~1.7x over bf16).  Weights are pre-scaled by W_SCALE on the host so they
# sit in e4m3's normal range; the activation reads descale for free via the
# ACT affine (scale=1/W_SCALE).
USE_FP8 = False
FP8 = mybir.dt.float8e4
W_SCALE = 32.0


def _build_core_program():
    nc = bacc.Bacc(
        "TRN2",
        target_bir_lowering=False,
        debug=False,
        enable_asserts=False,
        num_devices=N_CORES,
    )
    WDT = FP8 if USE_FP8 else BF16
    xq = nc.dram_tensor("xq", [TOK, DM], F32, kind="ExternalInput").ap()
    xk = nc.dram_tensor("xk", [TOK, DM], F32, kind="ExternalInput").ap()
    xv = nc.dram_tensor("xv", [TOK, DM], F32, kind="ExternalInput").ap()
    wk = nc.dram_tensor("wk", [DM, DM], WDT, kind="ExternalInput").ap()
    wv = nc.dram_tensor("wv", [DM, DM], WDT, kind="ExternalInput").ap()
    wored = nc.dram_tensor("wored", [H, DM], BF16, kind="ExternalInput").ap()
    out = nc.dram_tensor("out", [TOK, DM], F32, kind="ExternalOutput").ap()

    with tile.TileContext(nc) as tc:
        with ExitStack() as ctx:
            _emit(ctx, tc, xq, xk, xv, wk, wv, wored, out)

    nc.compile()
    return nc


def _emit(ctx, tc, xq, xk, xv, wk, wv, wored, out):
    nc = tc.nc

    const = ctx.enter_context(tc.tile_pool(name="const", bufs=1))
    wpool = ctx.enter_context(tc.tile_pool(name="w", bufs=1))
    xblk = ctx.enter_context(tc.tile_pool(name="xblk", bufs=3))
    xtp = ctx.enter_context(tc.tile_pool(name="xt", bufs=2))
    qp = ctx.enter_context(tc.tile_pool(name="q32", bufs=3))
    tmpb = ctx.enter_context(tc.tile_pool(name="tmpb", bufs=2))
    small = ctx.enter_context(tc.tile_pool(name="small", bufs=4))
    xresp = ctx.enter_context(tc.tile_pool(name="xres", bufs=2))
    outp = ctx.enter_context(tc.tile_pool(name="outp", bufs=3))
    ps_proj = ctx.enter_context(tc.tile_pool(name="ps_proj", bufs=2, space="PSUM"))
    ps_t = ctx.enter_context(tc.tile_pool(name="ps_t", bufs=1, space="PSUM"))
    ps_attn = ctx.enter_context(tc.tile_pool(name="ps_attn", bufs=1, space="PSUM"))
    ps_rt = ctx.enter_context(tc.tile_pool(name="ps_rt", bufs=1, space="PSUM"))

    ident = const.tile([128, 128], BF16)
    make_identity(nc, ident)

    # Constants for activation bias APs and the Newton iteration.
    cvals = [0.0, 1.0, 1.5]
    ctile = const.tile([128, len(cvals)], F32)
    for i, v in enumerate(cvals):
        nc.vector.memset(ctile[:, i : i + 1], v)
        nc.const_aps.aps[(F32, v)] = ctile[:, i : i + 1]
    c_1p5 = ctile[:, 2:3]

    xsrc = {"k": xk, "v": xv}
    state = {}

    def s_load(m, name):
        # Per-subtile SWDGE cast-load for the startup-critical first tiles:
        # small units let them finish first (the SDMA engines round-robin
        # between rings at packet granularity, so one huge early transfer
        # starves the critical weight load).
        t = xblk.tile([128, DM], BF16, tag=f"x{name}s")
        nc.gpsimd.dma_start(out=t, in_=xsrc[name][m * 128 : (m + 1) * 128, :])
        state[(name, "tok", m)] = t

    def s_load_pair(m, name):
        # Steady state uses 2-subtile transfers for HBM efficiency.
        t = xblk.tile([128, 2, DM], BF16, tag=f"x{name}p")
        nc.gpsimd.dma_start(
            out=t,
            in_=xsrc[name][m * 128 : (m + 2) * 128, :].rearrange(
                "(s p) j -> p s j", p=128
            ),
        )
        state[(name, "tok", m)] = t[:, 0, :]
        state[(name, "tok", m + 1)] = t[:, 1, :]

    def s_transpose(m, name):
        # 8 PE transposes of one subtile into one PSUM bank, one copy out.
        src = state.pop((name, "tok", m))
        pst = ps_t.tile([128, NCH, 128], BF16, tag="pst")
        for c in range(NCH):
            nc.tensor.transpose(
                pst[:, c, :], src[:, c * 128 : (c + 1) * 128], ident
            )
        xT = xtp.tile([128, NCH, 128], FP8 if USE_FP8 else BF16, tag=f"{name}T")
        if name == "k":
            nc.scalar.copy(xT, pst)
        else:
            nc.vector.tensor_scalar(
                out=xT, in0=pst, scalar1=0.0, scalar2=None, op0=OP.add
            )
        state[(name, "xT", m)] = xT

    def s_proj(m, name):
        xT = state.pop((name, "xT", m))
        w_sb = wk_sb if name == "k" else wv_sb
        p = ps_proj.tile([128, DM], F32, tag="proj")
        if USE_FP8:
            # DoubleRow: both operands supply chunk-PAIRS [128, 2, n].
            for c2 in range(NCH // 2):
                for h in range(2):
                    nc.tensor.matmul(
                        p[:, h * 512 : (h + 1) * 512],
                        lhsT=xT[:, 2 * c2 : 2 * c2 + 2, :],
                        rhs=w_sb[:, 2 * c2 : 2 * c2 + 2, h * 512 : (h + 1) * 512],
                        start=(c2 == 0),
                        stop=(c2 == NCH // 2 - 1),
                        perf_mode=mybir.MatmulPerfMode.DoubleRow,
                    )
        else:
            for c in range(NCH):
                for h in range(2):
                    nc.tensor.matmul(
                        p[:, h * 512 : (h + 1) * 512],
                        lhsT=xT[:, c, :],
                        rhs=w_sb[:, c, h * 512 : (h + 1) * 512],
                        start=(c == 0),
                        stop=(c == NCH - 1),
                    )
        state[(name, "ps", m)] = p

    def s_eluk(m):
        psk = state.pop(("k", "ps", m))
        psv = state.pop(("v", "ps", m))
        # elu(k)+1 == max(min(exp(k),1), k+1); bf16 intermediates for DVE 2x.
        # The 1/W_SCALE descale of the fp8 weight pre-scaling rides the free
        # ACT affine on every PSUM read.
        ds = 1.0 / W_SCALE if USE_FP8 else 1.0
        ek = tmpb.tile([128, DM], BF16, tag="ek")
        nc.scalar.activation(ek, psk, AF.Exp, scale=ds)
        k1 = tmpb.tile([128, DM], BF16, tag="k1")
        nc.scalar.activation(k1, psk, AF.Identity, bias=1.0, scale=ds)
        vb = tmpb.tile([128, DM], BF16, tag="vb")
        nc.scalar.mul(vb, psv, ds)
        kf = tmpb.tile([128, DM], BF16, tag="kf")
        nc.vector.scalar_tensor_tensor(
            out=kf, in0=ek, scalar=1.0, in1=k1, op0=OP.min, op1=OP.max
        )
        kv = tmpb.tile([128, DM], BF16, tag="kv")
        nc.vector.tensor_mul(kv, kf, vb)

        # TensorReduce has no 2x uop (1.2us/call); pre-fold 64->16 with two
        # bf16 tensor_tensor adds (which DO run 2x), then reduce the quarter.
        def headsum(src, tag):
            s3 = src.rearrange("p (h d) -> p h d", h=H)
            th = tmpb.tile([128, H, D // 2], BF16, tag=f"{tag}h")
            nc.vector.tensor_tensor(
                th, s3[:, :, 0 : D // 2], s3[:, :, D // 2 :], op=OP.add
            )
            tq = tmpb.tile([128, H, D // 4], BF16, tag=f"{tag}q")
            nc.vector.tensor_tensor(
                tq, th[:, :, 0 : D // 4], th[:, :, D // 4 :], op=OP.add
            )
            out = small.tile([128, H], F32, tag=f"{tag}s")
            nc.vector.reduce_sum(out, tq, axis=AX.X)
            return out

        ks = headsum(kf, "ks")
        wvs = headsum(kv, "wv")
        rk = small.tile([128, H], F32, tag="rk")
        nc.vector.reciprocal(rk, ks)
        r = small.tile([128, H], BF16, tag="r")
        nc.vector.tensor_mul(r, wvs, rk)
        state[("r", m)] = r

    def s_rT(m):
        r = state.pop(("r", m))
        rT_ps = ps_rt.tile([16, 128], BF16, tag="rt")
        nc.tensor.transpose(rT_ps, r, ident)
        rT = small.tile([16, 128], BF16, tag="rT")
        nc.scalar.copy(rT, rT_ps)
        state[("rT", m)] = rT

    def s_attn(m):
        rT = state.pop(("rT", m))
        ap_ps = ps_attn.tile([128, DM], F32, tag="attn")
        for h in range(2):
            nc.tensor.matmul(
                ap_ps[:, h * 512 : (h + 1) * 512],
                lhsT=rT,
                rhs=wo_sb[:, h * 512 : (h + 1) * 512],
                start=True,
                stop=True,
            )
        state[("attn", m)] = ap_ps

    def s_qload_pair(m):
        q32 = qp.tile([128, 2, DM], F32, tag="q32")
        nc.sync.dma_start(
            out=q32,
            in_=xq[m * 128 : (m + 2) * 128, :].rearrange("(s p) j -> p s j", p=128),
        )
        state[("q32", m)] = q32[:, 0, :]
        state[("q32", m + 1)] = q32[:, 1, :]

    def s_ln(m):
        ap_ps = state.pop(("attn", m))
        q32 = state.pop(("q32", m))
        # Residual add; row-sum (-> mean) rides along via accum_out.
        xres = xresp.tile([128, DM], F32, tag="xres")
        sx = small.tile([128, 2], F32, tag="sx")
        nc.vector.scalar_tensor_tensor(
            out=xres,
            in0=ap_ps,
            scalar=0.0,
            in1=q32,
            op0=OP.add,
            op1=OP.add,
            accum_out=sx[:, 0:1],
        )
        xsq = tmpb.tile([128, DM], BF16, tag="xsq")
        nc.scalar.activation(xsq, xres, AF.Square, accum_out=sx[:, 1:2])
        mv = small.tile([128, 2], F32, tag="mv")
        nc.vector.tensor_scalar(
            out=mv, in0=sx, scalar1=1.0 / DM, scalar2=None, op0=OP.mult
        )
        # rstd = rsqrt(var + eps): bit-trick seed + 2 Newton steps, DVE only
        # (the Sqrt/Ln ACT tables live in different table sets than Exp, and
        # a table-set switch costs ~2.7us -- never load anything but Exp).
        nwt = small.tile([128, 10], F32, tag="nwt")
        ve = nwt[:, 0:1]
        nc.vector.tensor_scalar(
            out=ve, in0=mv[:, 1:2], scalar1=EPS_LN, scalar2=None, op0=OP.add
        )
        mneg = nwt[:, 1:2]
        nc.vector.tensor_scalar(
            out=mneg, in0=mv[:, 0:1], scalar1=-1.0, scalar2=None, op0=OP.mult
        )
        v1 = nwt[:, 2:3]
        nc.vector.scalar_tensor_tensor(
            out=v1, in0=mneg, scalar=mv[:, 0:1], op0=OP.mult, in1=ve, op1=OP.add
        )
        hx = nwt[:, 3:4]
        nc.vector.tensor_scalar(
            out=hx, in0=v1, scalar1=0.5, scalar2=None, op0=OP.mult
        )
        sshift = nwt[:, 4:5].bitcast(I32)
        nc.vector.tensor_scalar(
            out=sshift,
            in0=v1.bitcast(I32),
            scalar1=1,
            scalar2=None,
            op0=OP.arith_shift_right,
        )
        # magic - s == (s ^ 0xffffffff) + (magic + 1)  (int32 wraparound)
        nc.vector.tensor_scalar(
            out=sshift, in0=sshift, scalar1=-1, scalar2=None, op0=OP.bitwise_xor
        )
        y = nwt[:, 5:6]
        nc.vector.tensor_scalar(
            out=y.bitcast(I32),
            in0=sshift,
            scalar1=RSQRT_MAGIC + 1,
            scalar2=None,
            op0=OP.add,
        )
        for it in range(2):
            yy = nwt[:, 6:7]
            nc.vector.tensor_mul(yy, y, y)
            t = nwt[:, 7:8]
            # t = yy*hx - 1.5 ; z = y*t = -Newton(y); two steps restore sign
            nc.vector.scalar_tensor_tensor(
                out=t, in0=yy, scalar=hx, in1=c_1p5, op0=OP.mult, op1=OP.subtract
            )
            z = nwt[:, 8 + it : 9 + it]
            nc.vector.tensor_mul(z, y, t)
            y = z
        rstd = y
        nb = nwt[:, 7:8]
        nc.vector.tensor_scalar(
            out=nb, in0=mv[:, 0:1], scalar1=-1.0, scalar2=rstd, op0=OP.mult, op1=OP.mult
        )
        # Outputs land pairwise in one tile; one 1 MiB store per pair.
        if m % 2 == 0:
            opair = outp.tile([128, 2, DM], F32, tag="o")
            state[("opair", m)] = opair
        else:
            opair = state.pop(("opair", m - 1))
        nc.scalar.activation(opair[:, m % 2, :], xres, AF.Identity, bias=nb, scale=rstd)
        if m % 2 == 1:
            nc.sync.dma_start(
                out=out[(m - 1) * 128 : (m + 1) * 128, :].rearrange(
                    "(s p) j -> p s j", p=128
                ),
                in_=opair,
            )

    # Prime with the startup-critical transfers first: subtile 0's k/v casts
    # and the weight loads, split across the three DMA-issuing rings
    # (gpsimd SWDGE / sync HWDGE / scalar HWDGE) so they share HBM evenly.
    WDT = FP8 if USE_FP8 else BF16
    s_load(0, "k")
    wk_sb = wpool.tile([128, NCH, DM], WDT, tag="wk")
    nc.sync.dma_start(out=wk_sb, in_=wk.rearrange("(c p) j -> p c j", p=128))
    s_load(0, "v")
    wv_sb = wpool.tile([128, NCH, DM], WDT, tag="wv")
    nc.scalar.dma_start(out=wv_sb, in_=wv.rearrange("(c p) j -> p c j", p=128))
    wo_sb = wpool.tile([H, DM], BF16, tag="wo")
    nc.sync.dma_start(out=wo_sb, in_=wored)
    s_load(1, "k")
    s_load(1, "v")
    s_load_pair(2, "k")
    s_load_pair(2, "v")

    # Software pipeline.  PE queue order per tick m:
    #   Tk(m+1) Pk(m) rT(m-1) Tv(m+1) Pv(m) attn(m-1)
    # -- transposes for the next subtile are interleaved between this
    # subtile's projections so the shared PSUM staging bank alternates k/v
    # with the drain copies hidden under projection matmuls.
    for m in range(-1, NSUB + 1):
        if m >= 0 and m % 2 == 0 and m + 4 < NSUB:
            s_load_pair(m + 4, "k")
            s_load_pair(m + 4, "v")
        if 0 <= m + 1 < NSUB:
            s_transpose(m + 1, "k")
        if 0 <= m < NSUB:
            s_proj(m, "k")
            if m % 2 == 0:
                s_qload_pair(m)
        if 0 <= m - 1 < NSUB:
            s_rT(m - 1)
        if 0 <= m + 1 < NSUB:
            s_transpose(m + 1, "v")
        if 0 <= m < NSUB:
            s_proj(m, "v")
        if 0 <= m - 1 < NSUB:
            s_attn(m - 1)
        if 0 <= m < NSUB:
            s_eluk(m)
        if 0 <= m - 1 < NSUB:
            s_ln(m - 1)


_NC_CACHE = None


def _get_program():
    global _NC_CACHE
    if _NC_CACHE is None:
        _NC_CACHE = _build_core_program()
    return _NC_CACHE


def _prep_weights(inputs):
    import ml_dtypes

    Wk = np.ascontiguousarray(np.asarray(inputs["Wk"], np.float32))
    Wv = np.ascontiguousarray(np.asarray(inputs["Wv"], np.float32))
    Wo = np.ascontiguousarray(np.asarray(inputs["Wo"], np.float32))
    if USE_FP8:
        wdt = ml_dtypes.float8_e4m3
        wk_c = (Wk * W_SCALE).astype(wdt)
        wv_c = (Wv * W_SCALE).astype(wdt)
    else:
        wk_c = Wk.astype(ml_dtypes.bfloat16)
        wv_c = Wv.astype(ml_dtypes.bfloat16)
    wored = Wo.reshape(H, D, DM).sum(axis=1).astype(ml_dtypes.bfloat16)
    return wk_c, wv_c, wored


def _make_in_maps(inputs):
    q = np.ascontiguousarray(np.asarray(inputs["query"], np.float32)).reshape(NTOK, DM)
    k = np.ascontiguousarray(np.asarray(inputs["key"], np.float32)).reshape(NTOK, DM)
    v = np.ascontiguousarray(np.asarray(inputs["value"], np.float32)).reshape(NTOK, DM)
    wk_c, wv_c, wored = _prep_weights(inputs)

    in_maps = []
    for i in range(N_CORES):
        sl = slice(i * TOK, (i + 1) * TOK)
        in_maps.append(
            {
                "xq": np.ascontiguousarray(q[sl]),
                "xk": np.ascontiguousarray(k[sl]),
                "xv": np.ascontiguousarray(v[sl]),
                "wk": wk_c,
                "wv": wv_c,
                "wored": wored,
            }
        )
    return in_maps


def kernel(**inputs) -> np.ndarray:
    nc = _get_program()
    in_maps = _make_in_maps(inputs)
    res = run_bass_kernel_spmd(nc, in_maps, core_ids=list(range(N_CORES)))
    full = np.concatenate([r["out"] for r in res.results], axis=0)
    return full.reshape(B, L, DM).astype(np.float32)
